# revision 27
# baseline (speedup 1.0000x reference)
"""Trainium2 Bass kernel for nn_CosmosAttentionBlock (B=4, N=2048, H=1024, I=4096).

Sharding: 8 cores = 4 batches x 2 query-halves (np.roll trick: softmax over
keys is permutation-invariant, so each core's 1024 query rows are rows
[0:1024] of its rolled x copy; K/V recomputed for all 2048 keys).

Attention GEMMs (QKV, scores, PV, proj) run as fp8(e4m3) DoubleRow matmuls
(2 K-tiles per PE pass = 2x the bf16/f32r rate). The MLP (fc1/fc2) runs in
bf16 (fp8 there costs ~3e-2 rel err vs the 2e-2 gate; measured via CPU sim).
LayerNorm + transposes in bf16. Host packs weights into the exact SBUF layout
(fp8/bf16, power-of-2 scaled to dodge e4m3 subnormals) so DMAs stream
contiguously. Scale bookkeeping:
  hn/z2 stored as 32*normalize(x) (32 folded into rstd)
  q,k stored as 32*q -> scores psum = 1024*(q.k), exp(scale=1/32768, bias=-2)
  v stored as 32*v; attn = exp(s/32 - 2) fp8; ctx = pv_psum/128 fp8
  denominator via ones(=8)-matmul makes recip*proj_psum exactly ctx_norm@Wp
  fc2 accumulates all K=4096 in one PSUM group; epilogue (psum + h) fused.
"""
import sys
from contextlib import ExitStack

sys.path.insert(0, "/opt/trn_rl_repo")
import numpy as np
import ml_dtypes
import concourse.bacc as bacc
import concourse.mybir as mybir
from concourse import tile
from concourse.bass_utils import run_bass_kernel_spmd

F32 = mybir.dt.float32
BF16 = mybir.dt.bfloat16
FP8 = mybir.dt.float8e4
AF = mybir.ActivationFunctionType
ALU = mybir.AluOpType
DR = mybir.MatmulPerfMode.DoubleRow
P = 128
H = 1024
I_FF = 4096
N_FULL = 2048
N_Q = 1024
EPS = 1e-6
SHIFT = -3.5          # exp(s/32 + SHIFT); cancels via denominator
CTX_DIV = 128.0       # ctx fp8 = psum / CTX_DIV
ONES_VAL = 4.0        # = SQ*SW/CTX_DIV (makes recip*proj_psum exact)
SW = 32.0             # fp8 weight scale (qkv/proj)
SA = 32.0             # activation scale for hn/z2
SQ = 16.0             # storage scale for q/k/v (overflow margin)

NP_FP8 = ml_dtypes.float8_e4m3
NP_BF16 = ml_dtypes.bfloat16

_CACHED_NC = None


def build():
    global _CACHED_NC
    if _CACHED_NC is not None:
        return _CACHED_NC
    nc = bacc.Bacc("TRN2", target_bir_lowering=False)

    x_d = nc.dram_tensor("x16", [P, 16, H], BF16, kind="ExternalInput")
    xr_d = nc.dram_tensor("xr16", [P, 8, H], BF16, kind="ExternalInput")
    wq_d = nc.dram_tensor("wq8", [P, 8, H], FP8, kind="ExternalInput")
    wk_d = nc.dram_tensor("wk8", [P, 8, H], FP8, kind="ExternalInput")
    wv_d = nc.dram_tensor("wv8", [P, 8, H], FP8, kind="ExternalInput")
    wp_d = nc.dram_tensor("wp8", [P, 8, H], FP8, kind="ExternalInput")
    wf1_d = nc.dram_tensor("wf116", [P, 8, I_FF], BF16, kind="ExternalInput")
    wf2_d = nc.dram_tensor("wf216", [P, 32, H], BF16, kind="ExternalInput")
    qb_d = nc.dram_tensor("qb32", [P, 8], F32, kind="ExternalInput")
    kb_d = nc.dram_tensor("kb32", [P, 8], F32, kind="ExternalInput")
    f1b_d = nc.dram_tensor("f1b", [P, 32], F32, kind="ExternalInput")
    ident_d = nc.dram_tensor("ident16", [P, P], BF16, kind="ExternalInput")
    onescol_d = nc.dram_tensor("onescol8", [P, 2], FP8, kind="ExternalInput")
    out_d = nc.dram_tensor("out", [N_Q, H], F32, kind="ExternalOutput")

    with tile.TileContext(nc, pool_alloc_mode="queue") as tc, ExitStack() as es:
        # LEFT (bottom->top): persistents | attnT | hnT ... later ffT
        const = es.enter_context(tc.tile_pool(name="const", bufs=1, side="left"))
        xr_pool = es.enter_context(tc.tile_pool(name="xr", bufs=1, side="left"))
        h_pool = es.enter_context(tc.tile_pool(name="h", bufs=1, side="left"))
        out_pool = es.enter_context(tc.tile_pool(name="outp", bufs=2, side="left"))
        lns = es.enter_context(tc.tile_pool(name="lns", bufs=16, side="left"))
        z1p = es.enter_context(tc.tile_pool(name="z1p", bufs=4, side="left"))
        # RIGHT (bottom->top): z2T | C(v,ctx,wp) | B(qT,kT,wv) | A(x,wq,wk)
        z2T_pool = es.enter_context(tc.tile_pool(name="z2T", bufs=1, side="right"))
        ps_mm = es.enter_context(tc.tile_pool(name="ps_mm", bufs=4, space="PSUM"))
        ps_tp = es.enter_context(tc.tile_pool(name="ps_tp", bufs=3, space="PSUM"))
        ps_sm = es.enter_context(tc.tile_pool(name="ps_sm", bufs=1, space="PSUM"))

        # ---- constants (small DMAs on scalar queue) ----
        ident = const.tile([P, P], BF16, tag="ident")
        nc.scalar.dma_start(ident[:], ident_d[:])
        ones_col = const.tile([P, 2], FP8, tag="ones_col")
        nc.scalar.dma_start(ones_col[:], onescol_d[:])
        qb_sb = const.tile([P, 8], F32, tag="qb")
        nc.scalar.dma_start(qb_sb[:], qb_d[:])
        kb_sb = const.tile([P, 8], F32, tag="kb")
        nc.scalar.dma_start(kb_sb[:], kb_d[:])
        f1b_sb = const.tile([P, 32], F32, tag="f1b")
        nc.scalar.dma_start(f1b_sb[:], f1b_d[:])
        eps_t = const.tile([P, 1], F32, tag="eps")
        nc.vector.memset(eps_t[:], EPS / 1024.0)
        shift_t = const.tile([P, 1], F32, tag="shift")
        nc.vector.memset(shift_t[:], SHIFT)
        recip_sb = const.tile([P, 8], F32, tag="recip")

        z2T = z2T_pool.tile([P, 8, N_Q], BF16)
        xr_sb = xr_pool.tile([P, 8, H], BF16)
        h_sb = h_pool.tile([P, 8, H], BF16)

        es_C = ExitStack()
        poolC = es_C.enter_context(tc.tile_pool(name="pC", bufs=1, side="right"))
        v_sb = poolC.tile([P, 16, H], FP8, tag="v")
        ctxT = poolC.tile([P, 8, N_Q], FP8, tag="ctx")
        wp_sb = poolC.tile([P, 8, H], FP8, tag="wp")
        nc.gpsimd.dma_start(wp_sb[:], wp_d[:])

        es_attn = ExitStack()
        attnT_pool = es_attn.enter_context(
            tc.tile_pool(name="attnT", bufs=1, side="left"))
        attnT = attnT_pool.tile([P, 16, N_Q], FP8)
        es_hnT = ExitStack()
        hnT_pool = es_hnT.enter_context(tc.tile_pool(name="hnT", bufs=1,
                                                     side="left"))
        hnT = hnT_pool.tile([P, 8, N_FULL], FP8)

        es_B = ExitStack()
        poolB = es_B.enter_context(tc.tile_pool(name="pB", bufs=1, side="right"))
        qT = poolB.tile([P, 8, N_Q], FP8, tag="qT")
        kT = poolB.tile([P, 8, N_FULL], FP8, tag="kT")
        wv_sb = poolB.tile([P, 8, H], FP8, tag="wv")
        nc.gpsimd.dma_start(wv_sb[:], wv_d[:])

        es_A = ExitStack()
        poolA = es_A.enter_context(tc.tile_pool(name="pA", bufs=1, side="right"))
        wq_sb = poolA.tile([P, 8, H], FP8, tag="wq")
        wk_sb = poolA.tile([P, 8, H], FP8, tag="wk")
        xch_pool = es_A.enter_context(tc.tile_pool(name="xch", bufs=4,
                                                   side="right"))
        x_chunks = []

        def x_load(c):
            xc = xch_pool.tile([P, 2, H], BF16, tag="xc")
            nc.sync.dma_start(xc[:], x_d[:, 2 * c:2 * c + 2, :])
            x_chunks.append(xc)

        x_load(0)
        nc.sync.dma_start(wq_sb[:], wq_d[:])
        x_load(1)
        nc.sync.dma_start(wk_sb[:], wk_d[:])
        for c in range(2, 4):
            x_load(c)
        nc.gpsimd.dma_start(xr_sb[:], xr_d[:])

        # ============ LN1 + transpose: x tile t -> hnT fp8 (32x) ============
        def ln1_tile(t):
            if t % 2 == 0 and t // 2 + 4 < 8:
                x_load(t // 2 + 4)
            stats = lns.tile([P, 2, 6], F32, tag="st")
            xs = x_chunks[t // 2][:, t % 2, :].rearrange("p (s f) -> p s f", s=2)
            nc.vector.bn_stats(out=stats[:, 0, :], in_=xs[:, 0, :])
            nc.vector.bn_stats(out=stats[:, 1, :], in_=xs[:, 1, :])
            mv = lns.tile([P, 2], F32, tag="mv")
            nc.vector.bn_aggr(out=mv[:], in_=stats[:])
            rstd = lns.tile([P, 1], F32, tag="rstd")
            nc.scalar.activation(out=rstd[:], in_=mv[:, 1:2], func=AF.Sqrt,
                                 bias=eps_t[:], scale=1.0 / 1024.0)
            nc.vector.reciprocal(out=rstd[:], in_=rstd[:])
            z1 = z1p.tile([P, H], BF16, tag="z1")
            nc.vector.tensor_scalar(out=z1[:], in0=x_chunks[t // 2][:, t % 2, :],
                                    scalar1=mv[:, 0:1], scalar2=rstd[:],
                                    op0=ALU.subtract, op1=ALU.mult)
            for hc in range(8):
                tp = ps_tp.tile([P, P], BF16, tag="tp")
                nc.tensor.transpose(tp[:], z1[:, hc * P:(hc + 1) * P], ident[:])
                dst = hnT[:, hc, t * P:(t + 1) * P]
                if hc % 2 == 0:
                    nc.scalar.copy(dst, tp[:])
                else:
                    nc.vector.tensor_copy(dst, tp[:])

        def gemm_dr(psum, lhs_tile, lhs_lo, rhs_tile, rhs_cols, kpairs,
                    first=True, last=True):
            for j in range(kpairs):
                nc.tensor.matmul(psum[:], lhs_tile[:, 2 * j:2 * j + 2, lhs_lo],
                                 rhs_tile[:, 2 * j:2 * j + 2, rhs_cols],
                                 start=(first and j == 0),
                                 stop=(last and j == kpairs - 1),
                                 perf_mode=DR)

        for t in range(4):
            ln1_tile(t)

        def q_gemm(nt):
            cols = slice(nt * 512, (nt + 1) * 512)
            for ho in range(8):
                psum = ps_mm.tile([P, 512], F32, tag="mm")
                gemm_dr(psum, wq_sb, slice(ho * P, (ho + 1) * P), hnT, cols, 4)
                nc.vector.tensor_scalar(out=qT[:, ho, cols], in0=psum[:],
                                        scalar1=SQ / (SA * SW),
                                        scalar2=qb_sb[:, ho:ho + 1],
                                        op0=ALU.mult, op1=ALU.add)

        q_gemm(0)
        for t in range(4, 8):
            ln1_tile(t)
        q_gemm(1)
        for t in range(8, 12):
            ln1_tile(t)

        def k_gemm(mt4):
            cols = slice(mt4 * 512, (mt4 + 1) * 512)
            for ho in range(8):
                psum = ps_mm.tile([P, 512], F32, tag="mm")
                gemm_dr(psum, wk_sb, slice(ho * P, (ho + 1) * P), hnT, cols, 4)
                nc.scalar.activation(out=kT[:, ho, cols], in_=psum[:],
                                     func=AF.Identity,
                                     bias=kb_sb[:, ho:ho + 1],
                                     scale=SQ / (SA * SW))

        k_gemm(0)
        k_gemm(1)
        for t in range(12, 16):
            ln1_tile(t)
        k_gemm(2)
        k_gemm(3)
        es_A.close()

        # ============ scores + exp & V gemm (interleaved) ============
        for mt in range(16):
            for half in range(2):
                cols = slice(half * 512, (half + 1) * 512)
                psum = ps_mm.tile([P, 512], F32, tag="mm")
                gemm_dr(psum, kT, slice(mt * P, (mt + 1) * P), qT, cols, 4)
                nc.scalar.activation(out=attnT[:, mt, cols], in_=psum[:],
                                     func=AF.Exp, bias=shift_t[:],
                                     scale=1.0 / (SQ * SQ * 32.0))
            for ot in range(2):
                ocols = slice(ot * 512, (ot + 1) * 512)
                psum = ps_mm.tile([P, 512], F32, tag="mm")
                gemm_dr(psum, hnT, slice(mt * P, (mt + 1) * P), wv_sb, ocols, 4)
                nc.vector.tensor_scalar_mul(v_sb[:, mt, ocols], psum[:],
                                            SQ / (SA * SW))
        es_hnT.close()
        es_B.close()

        # ============ denominators ============
        for nqc in range(8):
            dps = ps_sm.tile([P, 2], F32, tag="denom")
            for mt in range(16):
                nc.tensor.matmul(dps[:], attnT[:, mt, nqc * P:(nqc + 1) * P],
                                 ones_col[:], start=(mt == 0), stop=(mt == 15))
            nc.vector.reciprocal(out=recip_sb[:, nqc:nqc + 1], in_=dps[:, 0:1])

        # ============ PV -> ctxT; proj -> h; LN2 -> z2T ============
        def pv_ot(half, ot):
            cols = slice(half * 512, (half + 1) * 512)
            psum = ps_mm.tile([P, 512], F32, tag="mm")
            gemm_dr(psum, v_sb, slice(ot * P, (ot + 1) * P), attnT, cols, 8)
            if ot % 2 == 0:
                nc.scalar.activation(out=ctxT[:, ot, cols], in_=psum[:],
                                     func=AF.Copy, scale=1.0 / CTX_DIV)
            else:
                nc.vector.tensor_scalar_mul(ctxT[:, ot, cols], psum[:],
                                            1.0 / CTX_DIV)

        def pv_half(half):
            for ot in range(8):
                pv_ot(half, ot)

        def proj_ln2(nqt):
            for o2 in range(2):
                ocols = slice(o2 * 512, (o2 + 1) * 512)
                psum = ps_mm.tile([P, 512], F32, tag="mm")
                gemm_dr(psum, ctxT, slice(nqt * P, (nqt + 1) * P), wp_sb,
                        ocols, 4)
                nc.vector.scalar_tensor_tensor(
                    out=h_sb[:, nqt, ocols], in0=psum[:],
                    scalar=recip_sb[:, nqt:nqt + 1],
                    in1=xr_sb[:, nqt, ocols], op0=ALU.mult, op1=ALU.add)
            stats = lns.tile([P, 2, 6], F32, tag="st")
            hs = h_sb[:, nqt, :].rearrange("p (s f) -> p s f", s=2)
            nc.vector.bn_stats(out=stats[:, 0, :], in_=hs[:, 0, :])
            nc.vector.bn_stats(out=stats[:, 1, :], in_=hs[:, 1, :])
            mv = lns.tile([P, 2], F32, tag="mv")
            nc.vector.bn_aggr(out=mv[:], in_=stats[:])
            rstd = lns.tile([P, 1], F32, tag="rstd")
            nc.scalar.activation(out=rstd[:], in_=mv[:, 1:2], func=AF.Sqrt,
                                 bias=eps_t[:], scale=1.0 / 1024.0)
            nc.vector.reciprocal(out=rstd[:], in_=rstd[:])
            z2 = z1p.tile([P, H], BF16, tag="z1")
            nc.gpsimd.tensor_scalar(out=z2[:], in0=h_sb[:, nqt, :],
                                    scalar1=mv[:, 0:1], scalar2=rstd[:],
                                    op0=ALU.subtract, op1=ALU.mult)
            for hc in range(8):
                tp = ps_tp.tile([P, P], BF16, tag="tp")
                nc.tensor.transpose(tp[:], z2[:, hc * P:(hc + 1) * P], ident[:])
                dst = z2T[:, hc, nqt * P:(nqt + 1) * P]
                if hc % 2 == 0:
                    nc.scalar.copy(dst, tp[:])
                else:
                    nc.vector.tensor_copy(dst, tp[:])

        pv_half(0)
        for nqt in range(4):
            pv_ot(1, 2 * nqt)
            pv_ot(1, 2 * nqt + 1)
            proj_ln2(nqt)
        for nqt in range(4, 8):
            proj_ln2(nqt)
        es_attn.close()
        es_C.close()

        # ============ fc1 (bf16, wf1 streamed in 8 chunks) -> ffT ============
        es_ff = ExitStack()
        ffT_pool = es_ff.enter_context(tc.tile_pool(name="ffT", bufs=1,
                                                    side="left"))
        ffT = ffT_pool.tile([P, 32, N_Q], BF16)

        es_w2 = ExitStack()
        wf2c_pool = es_w2.enter_context(tc.tile_pool(name="wf2c", bufs=3,
                                                     side="right"))
        es_w1 = ExitStack()
        wf1c_pool = es_w1.enter_context(tc.tile_pool(name="wf1c", bufs=2,
                                                     side="right"))
        wf2cs = []
        for icr in range(8):
            wf1c = wf1c_pool.tile([P, 8, 512], BF16, tag="wf1c")
            nc.gpsimd.dma_start(wf1c[:, 0:4, :],
                                wf1_d[:, 0:4, icr * 512:(icr + 1) * 512])
            nc.scalar.dma_start(wf1c[:, 4:8, :],
                                wf1_d[:, 4:8, icr * 512:(icr + 1) * 512])
            if icr >= 4:  # prefetch wf2 quarters during fc1 tail
                oq = icr - 4
                wf2c = wf2c_pool.tile([P, 32, 256], BF16, tag="wf2c")
                nc.sync.dma_start(wf2c[:, 0:16, :],
                                  wf2_d[:, 0:16, oq * 256:(oq + 1) * 256])
                nc.gpsimd.dma_start(wf2c[:, 16:32, :],
                                    wf2_d[:, 16:32, oq * 256:(oq + 1) * 256])
                wf2cs.append(wf2c)
            for half in range(2):
                cols = slice(half * 512, (half + 1) * 512)
                for ic4 in range(4):
                    ic = icr * 4 + ic4
                    psum = ps_mm.tile([P, 512], F32, tag="mm")
                    for hi in range(8):
                        nc.tensor.matmul(psum[:],
                                         wf1c[:, hi, ic4 * P:(ic4 + 1) * P],
                                         z2T[:, hi, cols],
                                         start=(hi == 0), stop=(hi == 7))
                    nc.scalar.activation(out=ffT[:, ic, cols], in_=psum[:],
                                         func=AF.Gelu,
                                         bias=f1b_sb[:, ic:ic + 1],
                                         scale=1.0 / 32.0)

        es_w1.close()

        # ============ fc2 (bf16, wf2 quarter chunks) -> out ============
        oqs = [nc.sync, nc.gpsimd, nc.scalar]
        for oq in range(4):
            ocols = slice(oq * 256, (oq + 1) * 256)
            wf2c = wf2cs[oq]
            for nqt in range(8):
                psum = ps_mm.tile([P, 512], F32, tag="mm")
                ps = psum[:, 0:256]
                for t in range(32):
                    nc.tensor.matmul(ps, ffT[:, t, nqt * P:(nqt + 1) * P],
                                     wf2c[:, t, :],
                                     start=(t == 0), stop=(t == 31))
                out_sb = out_pool.tile([P, 256], F32, tag="out")
                nc.vector.scalar_tensor_tensor(
                    out=out_sb[:], in0=ps, scalar=1.0,
                    in1=h_sb[:, nqt, ocols], op0=ALU.mult, op1=ALU.add)
                oqs[(oq * 8 + nqt) % 3].dma_start(
                    out_d[nqt * P:(nqt + 1) * P, ocols], out_sb[:])
        es_w2.close()
        es_ff.close()

    nc.compile()
    _CACHED_NC = nc
    return nc


def _pack_w8(wT, scale, ktiles):
    """[K, M] -> [P, ktiles, M] fp8, rows k = c*P + p."""
    K, M = wT.shape
    assert K == ktiles * P
    w = (wT * scale).reshape(ktiles, P, M).transpose(1, 0, 2)
    return np.ascontiguousarray(np.clip(w, -240.0, 240.0)).astype(NP_FP8)


def _pack_w16(wT, ktiles):
    K, M = wT.shape
    w = wT.reshape(ktiles, P, M).transpose(1, 0, 2)
    return np.ascontiguousarray(w).astype(NP_BF16)


def _pack_tok(x):
    """[T, H] -> [P, T//P, H], rows t = c*P + p."""
    T, Hd = x.shape
    xx = x.reshape(T // P, P, Hd).transpose(1, 0, 2)
    return np.ascontiguousarray(xx).astype(NP_BF16)


def _host_prep(inputs):
    f = lambda a: np.ascontiguousarray(np.asarray(a, dtype=np.float32))
    x = f(inputs["x"])
    ln1_w, ln1_b = f(inputs["ln1_w"]), f(inputs["ln1_b"])
    ln2_w, ln2_b = f(inputs["ln2_w"]), f(inputs["ln2_b"])
    qkv_w, qkv_b = f(inputs["qkv_w"]), f(inputs["qkv_b"])
    proj_w, proj_b = f(inputs["proj_w"]), f(inputs["proj_b"])
    fc1_w, fc1_b = f(inputs["fc1_w"]), f(inputs["fc1_b"])
    fc2_w, fc2_b = f(inputs["fc2_w"]), f(inputs["fc2_b"])

    qkv_wf = qkv_w * ln1_w[None, :]
    qkv_bf = qkv_b + qkv_w @ ln1_b
    qb, kb, vb = qkv_bf[0:H], qkv_bf[H:2 * H], qkv_bf[2 * H:3 * H]
    fc1_wf = fc1_w * ln2_w[None, :]
    f1b = fc1_b + fc1_w @ ln2_b
    xr_row = proj_b + proj_w @ vb

    shared = {
        "wq8": _pack_w8(np.ascontiguousarray(qkv_wf[0:H].T), SW, 8),
        "wk8": _pack_w8(np.ascontiguousarray(qkv_wf[H:2 * H].T), SW, 8),
        "wv8": _pack_w8(np.ascontiguousarray(qkv_wf[2 * H:3 * H].T), SW, 8),
        "wp8": _pack_w8(np.ascontiguousarray(proj_w.T), SW, 8),
        "wf116": _pack_w16(np.ascontiguousarray(fc1_wf.T), 8),
        "wf216": _pack_w16(np.ascontiguousarray(fc2_w.T), 32),
        "qb32": np.ascontiguousarray((SQ * qb).reshape(8, P).T),
        "kb32": np.ascontiguousarray((SQ * kb).reshape(8, P).T),
        "f1b": np.ascontiguousarray(f1b.reshape(32, P).T),
        "ident16": np.eye(P, dtype=NP_BF16),
        "onescol8": np.full((P, 2), ONES_VAL, dtype=NP_FP8),
    }
    in_maps = []
    for c in range(8):
        b, half = c // 2, c % 2
        xb = x[b]
        if half == 1:
            xb = np.roll(xb, -N_Q, axis=0)
        xr = xb[0:N_Q] + xr_row[None, :]
        in_maps.append({"x16": _pack_tok(xb), "xr16": _pack_tok(xr), **shared})
    return in_maps


def _assemble(results):
    out = np.empty((4, N_FULL, H), dtype=np.float32)
    for c in range(8):
        b, half = c // 2, c % 2
        out[b, half * N_Q:(half + 1) * N_Q] = results[c]["out"]
    return out


def run(inputs, trace=False):
    nc = build()
    in_maps = _host_prep(inputs)
    res = run_bass_kernel_spmd(nc, in_maps, list(range(8)), trace=trace)
    out = _assemble(res.results)
    f2b = np.asarray(inputs["fc2_b"], dtype=np.float32)
    if np.any(f2b):
        out += f2b[None, None, :]
    return out, res


def kernel(**inputs) -> np.ndarray:
    out, _ = run(inputs)
    return out


# revision 28
# speedup vs baseline: 1.2385x; 1.2385x over previous
"""Trainium2 Bass kernel for nn_CosmosAttentionBlock (B=4, N=2048, H=1024, I=4096).

Sharding: 8 cores = 4 batches x 2 query-halves (np.roll trick: softmax over
keys is permutation-invariant, so each core's 1024 query rows are rows
[0:1024] of its rolled x copy; K/V recomputed for all 2048 keys).

Attention GEMMs (QKV, scores, PV, proj) run as fp8(e4m3) DoubleRow matmuls
(2 K-tiles per PE pass = 2x the bf16/f32r rate). The MLP (fc1/fc2) runs in
bf16 (fp8 there costs ~3e-2 rel err vs the 2e-2 gate; measured via CPU sim).
LayerNorm + transposes in bf16. Host packs weights into the exact SBUF layout
(fp8/bf16, power-of-2 scaled to dodge e4m3 subnormals) so DMAs stream
contiguously. Scale bookkeeping:
  hn/z2 stored as 32*normalize(x) (32 folded into rstd)
  q,k stored as 32*q -> scores psum = 1024*(q.k), exp(scale=1/32768, bias=-2)
  v stored as 32*v; attn = exp(s/32 - 2) fp8; ctx = pv_psum/128 fp8
  denominator via ones(=8)-matmul makes recip*proj_psum exactly ctx_norm@Wp
  fc2 accumulates all K=4096 in one PSUM group; epilogue (psum + h) fused.
"""
import sys
from contextlib import ExitStack

sys.path.insert(0, "/opt/trn_rl_repo")
import numpy as np
import ml_dtypes
import concourse.bacc as bacc
import concourse.mybir as mybir
from concourse import tile
from concourse.bass_utils import run_bass_kernel_spmd

F32 = mybir.dt.float32
BF16 = mybir.dt.bfloat16
FP8 = mybir.dt.float8e4
AF = mybir.ActivationFunctionType
ALU = mybir.AluOpType
DR = mybir.MatmulPerfMode.DoubleRow
P = 128
H = 1024
I_FF = 4096
N_FULL = 2048
N_Q = 1024
EPS = 1e-6
SHIFT = -3.5          # exp(s/32 + SHIFT); cancels via denominator
CTX_DIV = 128.0       # ctx fp8 = psum / CTX_DIV
ONES_VAL = 4.0        # = SQ*SW/CTX_DIV (makes recip*proj_psum exact)
SW = 32.0             # fp8 weight scale (qkv/proj)
SA = 32.0             # activation scale for hn/z2
SQ = 16.0             # storage scale for q/k/v (overflow margin)

NP_FP8 = ml_dtypes.float8_e4m3
NP_BF16 = ml_dtypes.bfloat16

_CACHED_NC = None


def build():
    global _CACHED_NC
    if _CACHED_NC is not None:
        return _CACHED_NC
    nc = bacc.Bacc("TRN2", target_bir_lowering=False)

    x_d = nc.dram_tensor("x16", [P, 16, H], BF16, kind="ExternalInput")
    xr_d = nc.dram_tensor("xr16", [P, 8, H], BF16, kind="ExternalInput")
    wq_d = nc.dram_tensor("wq8", [P, 8, H], FP8, kind="ExternalInput")
    wk_d = nc.dram_tensor("wk8", [P, 8, H], FP8, kind="ExternalInput")
    wv_d = nc.dram_tensor("wv8", [P, 8, H], FP8, kind="ExternalInput")
    wp_d = nc.dram_tensor("wp8", [P, 8, H], FP8, kind="ExternalInput")
    wf1_d = nc.dram_tensor("wf116", [P, 8, I_FF], BF16, kind="ExternalInput")
    wf2_d = nc.dram_tensor("wf216", [P, 32, H], BF16, kind="ExternalInput")
    qb_d = nc.dram_tensor("qb32", [P, 8], F32, kind="ExternalInput")
    kb_d = nc.dram_tensor("kb32", [P, 8], F32, kind="ExternalInput")
    f1b_d = nc.dram_tensor("f1b", [P, 32], F32, kind="ExternalInput")
    ident_d = nc.dram_tensor("ident16", [P, P], BF16, kind="ExternalInput")
    onescol_d = nc.dram_tensor("onescol8", [P, 2], FP8, kind="ExternalInput")
    out_d = nc.dram_tensor("out", [N_Q, H], F32, kind="ExternalOutput")

    with tile.TileContext(nc, pool_alloc_mode="queue") as tc, ExitStack() as es:
        # LEFT (bottom->top): persistents | attnT | hnT ... later ffT
        const = es.enter_context(tc.tile_pool(name="const", bufs=1, side="left"))
        xr_pool = es.enter_context(tc.tile_pool(name="xr", bufs=1, side="left"))
        h_pool = es.enter_context(tc.tile_pool(name="h", bufs=1, side="left"))
        out_pool = es.enter_context(tc.tile_pool(name="outp", bufs=2, side="left"))
        lns = es.enter_context(tc.tile_pool(name="lns", bufs=16, side="left"))
        z1p = es.enter_context(tc.tile_pool(name="z1p", bufs=4, side="left"))
        # RIGHT (bottom->top): z2T | C(v,ctx,wp) | B(qT,kT,wv) | A(x,wq,wk)
        z2T_pool = es.enter_context(tc.tile_pool(name="z2T", bufs=1, side="right"))
        ps_mm = es.enter_context(tc.tile_pool(name="ps_mm", bufs=4, space="PSUM"))
        ps_tp = es.enter_context(tc.tile_pool(name="ps_tp", bufs=3, space="PSUM"))
        ps_sm = es.enter_context(tc.tile_pool(name="ps_sm", bufs=1, space="PSUM"))

        # ---- constants (small DMAs on scalar queue) ----
        ident = const.tile([P, P], BF16, tag="ident")
        nc.scalar.dma_start(ident[:], ident_d[:])
        ones_col = const.tile([P, 2], FP8, tag="ones_col")
        nc.scalar.dma_start(ones_col[:], onescol_d[:])
        qb_sb = const.tile([P, 8], F32, tag="qb")
        nc.scalar.dma_start(qb_sb[:], qb_d[:])
        kb_sb = const.tile([P, 8], F32, tag="kb")
        nc.scalar.dma_start(kb_sb[:], kb_d[:])
        f1b_sb = const.tile([P, 32], F32, tag="f1b")
        nc.scalar.dma_start(f1b_sb[:], f1b_d[:])
        eps_t = const.tile([P, 1], F32, tag="eps")
        nc.vector.memset(eps_t[:], EPS / 1024.0)
        shift_t = const.tile([P, 1], F32, tag="shift")
        nc.vector.memset(shift_t[:], SHIFT)
        recip_sb = const.tile([P, 8], F32, tag="recip")

        z2T = z2T_pool.tile([P, 8, N_Q], BF16)
        xr_sb = xr_pool.tile([P, 8, H], BF16)
        h_sb = h_pool.tile([P, 8, H], BF16)

        es_C = ExitStack()
        poolC = es_C.enter_context(tc.tile_pool(name="pC", bufs=1, side="right"))
        v_sb = poolC.tile([P, 16, H], FP8, tag="v")
        ctxT = poolC.tile([P, 8, N_Q], FP8, tag="ctx")
        wp_sb = poolC.tile([P, 8, H], FP8, tag="wp")
        nc.gpsimd.dma_start(wp_sb[:], wp_d[:])

        es_attn = ExitStack()
        attnT_pool = es_attn.enter_context(
            tc.tile_pool(name="attnT", bufs=1, side="left"))
        attnT = attnT_pool.tile([P, 16, N_Q], FP8)
        es_hnT = ExitStack()
        hnT_pool = es_hnT.enter_context(tc.tile_pool(name="hnT", bufs=1,
                                                     side="left"))
        hnT = hnT_pool.tile([P, 8, N_FULL], FP8)

        es_B = ExitStack()
        poolB = es_B.enter_context(tc.tile_pool(name="pB", bufs=1, side="right"))
        qT = poolB.tile([P, 8, N_Q], FP8, tag="qT")
        kT = poolB.tile([P, 8, N_FULL], FP8, tag="kT")
        wv_sb = poolB.tile([P, 8, H], FP8, tag="wv")
        nc.gpsimd.dma_start(wv_sb[:], wv_d[:])

        es_A = ExitStack()
        poolA = es_A.enter_context(tc.tile_pool(name="pA", bufs=1, side="right"))
        wq_sb = poolA.tile([P, 8, H], FP8, tag="wq")
        wk_sb = poolA.tile([P, 8, H], FP8, tag="wk")
        xch_pool = es_A.enter_context(tc.tile_pool(name="xch", bufs=4,
                                                   side="right"))
        x_chunks = []

        def x_load(c):
            xc = xch_pool.tile([P, 2, H], BF16, tag="xc")
            nc.sync.dma_start(xc[:], x_d[:, 2 * c:2 * c + 2, :])
            x_chunks.append(xc)

        x_load(0)
        nc.sync.dma_start(wq_sb[:], wq_d[:])
        x_load(1)
        nc.sync.dma_start(wk_sb[:], wk_d[:])
        for c in range(2, 4):
            x_load(c)
        nc.gpsimd.dma_start(xr_sb[:], xr_d[:])

        # ============ LN1 + transpose: x tile t -> hnT fp8 (32x) ============
        def ln1_tile(t):
            if t % 2 == 0 and t // 2 + 4 < 8:
                x_load(t // 2 + 4)
            stats = lns.tile([P, 2, 6], F32, tag="st")
            xs = x_chunks[t // 2][:, t % 2, :].rearrange("p (s f) -> p s f", s=2)
            nc.vector.bn_stats(out=stats[:, 0, :], in_=xs[:, 0, :])
            nc.vector.bn_stats(out=stats[:, 1, :], in_=xs[:, 1, :])
            mv = lns.tile([P, 2], F32, tag="mv")
            nc.vector.bn_aggr(out=mv[:], in_=stats[:])
            rstd = lns.tile([P, 1], F32, tag="rstd")
            nc.scalar.activation(out=rstd[:], in_=mv[:, 1:2], func=AF.Sqrt,
                                 bias=eps_t[:], scale=1.0 / 1024.0)
            nc.vector.reciprocal(out=rstd[:], in_=rstd[:])
            z1 = z1p.tile([P, H], BF16, tag="z1")
            nc.vector.tensor_scalar(out=z1[:], in0=x_chunks[t // 2][:, t % 2, :],
                                    scalar1=mv[:, 0:1], scalar2=rstd[:],
                                    op0=ALU.subtract, op1=ALU.mult)
            for hc in range(8):
                tp = ps_tp.tile([P, P], BF16, tag="tp")
                nc.tensor.transpose(tp[:], z1[:, hc * P:(hc + 1) * P], ident[:])
                dst = hnT[:, hc, t * P:(t + 1) * P]
                if hc % 2 == 0:
                    nc.scalar.copy(dst, tp[:])
                else:
                    nc.vector.tensor_copy(dst, tp[:])

        def gemm_dr(psum, lhs_tile, lhs_lo, rhs_tile, rhs_cols, kpairs,
                    first=True, last=True):
            for j in range(kpairs):
                nc.tensor.matmul(psum[:], lhs_tile[:, 2 * j:2 * j + 2, lhs_lo],
                                 rhs_tile[:, 2 * j:2 * j + 2, rhs_cols],
                                 start=(first and j == 0),
                                 stop=(last and j == kpairs - 1),
                                 perf_mode=DR)

        for t in range(4):
            ln1_tile(t)

        def q_gemm(nt):
            cols = slice(nt * 512, (nt + 1) * 512)
            for ho in range(8):
                psum = ps_mm.tile([P, 512], F32, tag="mm")
                gemm_dr(psum, wq_sb, slice(ho * P, (ho + 1) * P), hnT, cols, 4)
                nc.vector.tensor_scalar(out=qT[:, ho, cols], in0=psum[:],
                                        scalar1=SQ / (SA * SW),
                                        scalar2=qb_sb[:, ho:ho + 1],
                                        op0=ALU.mult, op1=ALU.add)

        q_gemm(0)
        for t in range(4, 8):
            ln1_tile(t)
        q_gemm(1)
        for t in range(8, 12):
            ln1_tile(t)

        def k_gemm(mt4):
            cols = slice(mt4 * 512, (mt4 + 1) * 512)
            for ho in range(8):
                psum = ps_mm.tile([P, 512], F32, tag="mm")
                gemm_dr(psum, wk_sb, slice(ho * P, (ho + 1) * P), hnT, cols, 4)
                nc.scalar.activation(out=kT[:, ho, cols], in_=psum[:],
                                     func=AF.Identity,
                                     bias=kb_sb[:, ho:ho + 1],
                                     scale=SQ / (SA * SW))

        k_gemm(0)
        k_gemm(1)
        for t in range(12, 16):
            ln1_tile(t)
        k_gemm(2)
        k_gemm(3)
        es_A.close()

        # ============ scores + exp & V gemm (interleaved) ============
        for mt in range(16):
            for half in range(2):
                cols = slice(half * 512, (half + 1) * 512)
                psum = ps_mm.tile([P, 512], F32, tag="mm")
                gemm_dr(psum, kT, slice(mt * P, (mt + 1) * P), qT, cols, 4)
                nc.scalar.activation(out=attnT[:, mt, cols], in_=psum[:],
                                     func=AF.Exp, bias=shift_t[:],
                                     scale=1.0 / (SQ * SQ * 32.0))
            for ot in range(2):
                ocols = slice(ot * 512, (ot + 1) * 512)
                psum = ps_mm.tile([P, 512], F32, tag="mm")
                gemm_dr(psum, hnT, slice(mt * P, (mt + 1) * P), wv_sb, ocols, 4)
                nc.vector.tensor_scalar_mul(v_sb[:, mt, ocols], psum[:],
                                            SQ / (SA * SW))
        es_hnT.close()
        es_B.close()

        # ============ denominators ============
        for nqc in range(8):
            dps = ps_sm.tile([P, 2], F32, tag="denom")
            for mt in range(16):
                nc.tensor.matmul(dps[:], attnT[:, mt, nqc * P:(nqc + 1) * P],
                                 ones_col[:], start=(mt == 0), stop=(mt == 15))
            nc.vector.reciprocal(out=recip_sb[:, nqc:nqc + 1], in_=dps[:, 0:1])

        # ============ PV -> ctxT; proj -> h; LN2 -> z2T ============
        def pv_ot(half, ot):
            cols = slice(half * 512, (half + 1) * 512)
            psum = ps_mm.tile([P, 512], F32, tag="mm")
            gemm_dr(psum, v_sb, slice(ot * P, (ot + 1) * P), attnT, cols, 8)
            if ot % 2 == 0:
                nc.scalar.activation(out=ctxT[:, ot, cols], in_=psum[:],
                                     func=AF.Copy, scale=1.0 / CTX_DIV)
            else:
                nc.vector.tensor_scalar_mul(ctxT[:, ot, cols], psum[:],
                                            1.0 / CTX_DIV)

        def pv_half(half):
            for ot in range(8):
                pv_ot(half, ot)

        def proj_ln2(nqt):
            for o2 in range(2):
                ocols = slice(o2 * 512, (o2 + 1) * 512)
                psum = ps_mm.tile([P, 512], F32, tag="mm")
                gemm_dr(psum, ctxT, slice(nqt * P, (nqt + 1) * P), wp_sb,
                        ocols, 4)
                nc.vector.scalar_tensor_tensor(
                    out=h_sb[:, nqt, ocols], in0=psum[:],
                    scalar=recip_sb[:, nqt:nqt + 1],
                    in1=xr_sb[:, nqt, ocols], op0=ALU.mult, op1=ALU.add)
            stats = lns.tile([P, 2, 6], F32, tag="st")
            hs = h_sb[:, nqt, :].rearrange("p (s f) -> p s f", s=2)
            nc.vector.bn_stats(out=stats[:, 0, :], in_=hs[:, 0, :])
            nc.vector.bn_stats(out=stats[:, 1, :], in_=hs[:, 1, :])
            mv = lns.tile([P, 2], F32, tag="mv")
            nc.vector.bn_aggr(out=mv[:], in_=stats[:])
            rstd = lns.tile([P, 1], F32, tag="rstd")
            nc.scalar.activation(out=rstd[:], in_=mv[:, 1:2], func=AF.Sqrt,
                                 bias=eps_t[:], scale=1.0 / 1024.0)
            nc.vector.reciprocal(out=rstd[:], in_=rstd[:])
            z2 = z1p.tile([P, H], BF16, tag="z1")
            nc.vector.tensor_scalar(out=z2[:], in0=h_sb[:, nqt, :],
                                    scalar1=mv[:, 0:1], scalar2=rstd[:],
                                    op0=ALU.subtract, op1=ALU.mult)
            for hc in range(8):
                tp = ps_tp.tile([P, P], BF16, tag="tp")
                nc.tensor.transpose(tp[:], z2[:, hc * P:(hc + 1) * P], ident[:])
                dst = z2T[:, hc, nqt * P:(nqt + 1) * P]
                if hc % 2 == 0:
                    nc.scalar.copy(dst, tp[:])
                else:
                    nc.vector.tensor_copy(dst, tp[:])

        pv_half(0)
        for nqt in range(4):
            pv_ot(1, 2 * nqt)
            pv_ot(1, 2 * nqt + 1)
            proj_ln2(nqt)
        for nqt in range(4, 8):
            proj_ln2(nqt)
        es_attn.close()
        es_C.close()

        # ============ fc1 (bf16, wf1 streamed in 8 chunks) -> ffT ============
        es_ff = ExitStack()
        ffT_pool = es_ff.enter_context(tc.tile_pool(name="ffT", bufs=1,
                                                    side="left"))
        ffT = ffT_pool.tile([P, 32, N_Q], BF16)

        es_w2 = ExitStack()
        wf2c_pool = es_w2.enter_context(tc.tile_pool(name="wf2c", bufs=3,
                                                     side="right"))
        es_w1 = ExitStack()
        wf1c_pool = es_w1.enter_context(tc.tile_pool(name="wf1c", bufs=2,
                                                     side="right"))
        wf2cs = []
        for icr in range(8):
            wf1c = wf1c_pool.tile([P, 8, 512], BF16, tag="wf1c")
            nc.gpsimd.dma_start(wf1c[:, 0:4, :],
                                wf1_d[:, 0:4, icr * 512:(icr + 1) * 512])
            nc.scalar.dma_start(wf1c[:, 4:8, :],
                                wf1_d[:, 4:8, icr * 512:(icr + 1) * 512])
            if icr >= 4:  # prefetch wf2 quarters during fc1 tail
                oq = icr - 4
                wf2c = wf2c_pool.tile([P, 32, 256], BF16, tag="wf2c")
                nc.sync.dma_start(wf2c[:, 0:16, :],
                                  wf2_d[:, 0:16, oq * 256:(oq + 1) * 256])
                nc.gpsimd.dma_start(wf2c[:, 16:32, :],
                                    wf2_d[:, 16:32, oq * 256:(oq + 1) * 256])
                wf2cs.append(wf2c)
            for half in range(2):
                cols = slice(half * 512, (half + 1) * 512)
                for ic4 in range(4):
                    ic = icr * 4 + ic4
                    psum = ps_mm.tile([P, 512], F32, tag="mm")
                    for hi in range(8):
                        nc.tensor.matmul(psum[:],
                                         wf1c[:, hi, ic4 * P:(ic4 + 1) * P],
                                         z2T[:, hi, cols],
                                         start=(hi == 0), stop=(hi == 7))
                    nc.scalar.activation(out=ffT[:, ic, cols], in_=psum[:],
                                         func=AF.Gelu,
                                         bias=f1b_sb[:, ic:ic + 1],
                                         scale=1.0 / 32.0)

        es_w1.close()

        # ============ fc2 (bf16, wf2 quarter chunks) -> out ============
        oqs = [nc.sync, nc.gpsimd, nc.scalar]
        for oq in range(4):
            ocols = slice(oq * 256, (oq + 1) * 256)
            wf2c = wf2cs[oq]
            for nqt in range(8):
                psum = ps_mm.tile([P, 512], F32, tag="mm")
                ps = psum[:, 0:256]
                for t in range(32):
                    nc.tensor.matmul(ps, ffT[:, t, nqt * P:(nqt + 1) * P],
                                     wf2c[:, t, :],
                                     start=(t == 0), stop=(t == 31))
                out_sb = out_pool.tile([P, 256], F32, tag="out")
                nc.vector.scalar_tensor_tensor(
                    out=out_sb[:], in0=ps, scalar=1.0,
                    in1=h_sb[:, nqt, ocols], op0=ALU.mult, op1=ALU.add)
                oqs[(oq * 8 + nqt) % 3].dma_start(
                    out_d[nqt * P:(nqt + 1) * P, ocols], out_sb[:])
        es_w2.close()
        es_ff.close()

    nc.compile()
    _CACHED_NC = nc
    return nc


def _pack_w8(wT, scale, ktiles):
    """[K, M] -> [P, ktiles, M] fp8, rows k = c*P + p."""
    K, M = wT.shape
    assert K == ktiles * P
    w = (wT * scale).reshape(ktiles, P, M).transpose(1, 0, 2)
    return np.ascontiguousarray(np.clip(w, -240.0, 240.0)).astype(NP_FP8)


def _pack_w16(wT, ktiles):
    K, M = wT.shape
    w = wT.reshape(ktiles, P, M).transpose(1, 0, 2)
    return np.ascontiguousarray(w).astype(NP_BF16)


def _pack_tok(x):
    """[T, H] -> [P, T//P, H], rows t = c*P + p."""
    T, Hd = x.shape
    xx = x.reshape(T // P, P, Hd).transpose(1, 0, 2)
    return np.ascontiguousarray(xx).astype(NP_BF16)


def _host_prep(inputs):
    f = lambda a: np.ascontiguousarray(np.asarray(a, dtype=np.float32))
    x = f(inputs["x"])
    ln1_w, ln1_b = f(inputs["ln1_w"]), f(inputs["ln1_b"])
    ln2_w, ln2_b = f(inputs["ln2_w"]), f(inputs["ln2_b"])
    qkv_w, qkv_b = f(inputs["qkv_w"]), f(inputs["qkv_b"])
    proj_w, proj_b = f(inputs["proj_w"]), f(inputs["proj_b"])
    fc1_w, fc1_b = f(inputs["fc1_w"]), f(inputs["fc1_b"])
    fc2_w, fc2_b = f(inputs["fc2_w"]), f(inputs["fc2_b"])

    qkv_wf = qkv_w * ln1_w[None, :]
    qkv_bf = qkv_b + qkv_w @ ln1_b
    qb, kb, vb = qkv_bf[0:H], qkv_bf[H:2 * H], qkv_bf[2 * H:3 * H]
    fc1_wf = fc1_w * ln2_w[None, :]
    f1b = fc1_b + fc1_w @ ln2_b
    xr_row = proj_b + proj_w @ vb

    shared = {
        "wq8": _pack_w8(np.ascontiguousarray(qkv_wf[0:H].T), SW, 8),
        "wk8": _pack_w8(np.ascontiguousarray(qkv_wf[H:2 * H].T), SW, 8),
        "wv8": _pack_w8(np.ascontiguousarray(qkv_wf[2 * H:3 * H].T), SW, 8),
        "wp8": _pack_w8(np.ascontiguousarray(proj_w.T), SW, 8),
        "wf116": _pack_w16(np.ascontiguousarray(fc1_wf.T), 8),
        "wf216": _pack_w16(np.ascontiguousarray(fc2_w.T), 32),
        "qb32": np.ascontiguousarray((SQ * qb).reshape(8, P).T),
        "kb32": np.ascontiguousarray((SQ * kb).reshape(8, P).T),
        "f1b": np.ascontiguousarray(f1b.reshape(32, P).T),
        "ident16": np.eye(P, dtype=NP_BF16),
        "onescol8": np.full((P, 2), ONES_VAL, dtype=NP_FP8),
    }
    in_maps = []
    for c in range(8):
        b, half = c // 2, c % 2
        xb = x[b]
        if half == 1:
            xb = np.roll(xb, -N_Q, axis=0)
        xr = xb[0:N_Q] + xr_row[None, :]
        in_maps.append({"x16": _pack_tok(xb), "xr16": _pack_tok(xr), **shared})
    return in_maps


def _assemble(results):
    out = np.empty((4, N_FULL, H), dtype=np.float32)
    for c in range(8):
        b, half = c // 2, c % 2
        out[b, half * N_Q:(half + 1) * N_Q] = results[c]["out"]
    return out


def run(inputs, trace=False):
    nc = build()
    in_maps = _host_prep(inputs)
    res = run_bass_kernel_spmd(nc, in_maps, list(range(8)), trace=trace)
    out = _assemble(res.results)
    f2b = np.asarray(inputs["fc2_b"], dtype=np.float32)
    if np.any(f2b):
        out += f2b[None, None, :]
    return out, res


def kernel(**inputs) -> np.ndarray:
    out, _ = run(inputs)
    return out
